# revision 45
# baseline (speedup 1.0000x reference)
"""SigLIP2 attention block on 8 TRN2 NeuronCores.

Strategy: data-parallel over batch (B=8 -> 1 batch element per core, no
collectives). All weights pre-transposed/pre-tiled + pre-cast to bf16 on the
host so the on-chip kernel is a pure matmul + softmax pipeline. Matmul time
on the PE is proportional to the OUTPUT FREE SIZE only, so every matmul is
oriented with the big dims on partitions and the small dims on free:

  per core (batch b):
    v:      psum[s,j] = hT[d,s].T @ v_wT[d,j]           (free 288)
    qkv:    q/k psum[j,s] = qk_wT[d,j].T @ hT[d,s]      (free 512, j-major)
    rope:   per-head redistribute (partition-shifted SBUF DMA on gpsimd) +
            rot-half via shifted copies, then q' = q*cosT + rot(q)*sinT_signed
            on DVE (in place)
    attn:   scores_T[ks,qs] = k'h[hd,ks].T @ q'h[hd,qs]  (K=72, free 512)
            exp on ACT (scale=1/sqrt(72), no max-subtract: |scores| is O(1))
            PV: attn[qs,hd+1] accumulated over ks tiles with ex[ks,qs] as the
            stationary operand (free 73!). vpad has a ones column at 72 so
            the softmax denominator lands in free col 72; normalize with
            reciprocal + tensor_scalar (per-partition scalar) on DVE into
            qs-major att_s tiles.
    trans:  att_s[qs,f] -> attnp[f,qs] via PE identity-matmul transposes
            (4 qt chunks per psum tile, one DVE copy per group)
    proj:   out[s,e] = attnp[f,s].T @ proj_wT[f,e] (free 384), split into
            P1 (f-tiles 0..4, staged to SBUF bf16, emitted inside head 15's
            scores->PV window to fill the ACT-bound stretch) and P2
            (f-tiles 5..8; P1 added back in the DVE output copy)
  proj_b added on host (linear); qkv_b is all-zero in this problem (asserted).

  Schedule: pair-0 qk tiles interleave per hT d-slab during the DMA-paced
  startup; v runs right after pair 0 (hides head 0's redistribute+rope
  latency); later pairs alternate with the heads they complete. Bulk weight
  loads issue from the ACT queue (idle until the first exp) because SP's
  DMA-issue rate is the startup bottleneck.
"""

import os
import sys
import numpy as np

sys.path.insert(0, "/opt/trn_rl_repo")

B, S, D = 8, 1024, 1152
H, HD = 16, 72
HHD = HD // 2  # 36
NQK = 2 * D    # 2304 q+k rows
P = 128
NCORES = 8
SCALE = float(HD) ** -0.5

ND = D // P      # 9 d tiles
NS = S // P      # 8 s tiles
NJQK = NQK // P  # 18 qk j tiles
VP = HD + 1      # 73: head dim + denominator col
VPADW = H * VP   # 1168

_CACHE = {}


def _build():
    import concourse.bass as bass
    import concourse.bacc as bacc
    import concourse.mybir as mybir
    from concourse import tile
    from concourse.masks import make_identity

    bf16 = mybir.dt.bfloat16
    f32 = mybir.dt.float32

    nc = bacc.Bacc(None)

    # host-pretiled inputs (see prep_in_maps)
    hT_d = nc.declare_dram_parameter("hT", [P, ND * S], bf16, isOutput=False)
    cosT_d = nc.declare_dram_parameter("cosT", [HD, S], bf16, isOutput=False)
    sinT_d = nc.declare_dram_parameter("sinT", [HD, S], bf16, isOutput=False)
    qkwT_d = nc.declare_dram_parameter("qkwT", [P, NJQK * D], bf16,
                                       isOutput=False)
    vwT_d = nc.declare_dram_parameter("vwT", [P, ND * D], bf16, isOutput=False)
    pwT_d = nc.declare_dram_parameter("pwT", [P, ND * D], bf16, isOutput=False)
    out_d = nc.declare_dram_parameter("out", [S, D], f32, isOutput=True)

    VCH = 4 * HD * ND  # 2592 cols per vw hc-chunk

    with tile.TileContext(nc) as tc:
        with (
            tc.tile_pool(name="persist", bufs=1) as pp,
            tc.tile_pool(name="wstream", bufs=3) as wsp,
            tc.tile_pool(name="work", bufs=2) as wp,
            tc.tile_pool(name="expp", bufs=10) as ep,
            tc.tile_pool(name="psp", bufs=2, space="PSUM") as psp,
        ):
            # ---- resident allocations ----
            hT = pp.tile([P, ND * S], bf16, tag="hT", name="hT")
            vwT = pp.tile([P, ND * D], bf16, tag="vwT", name="vwT")
            pwT = pp.tile([P, ND * D], bf16, tag="pwT", name="pwT")
            cosT = pp.tile([P, S], bf16, tag="cosT", name="cosT")
            sinT = pp.tile([P, S], bf16, tag="sinT", name="sinT")
            ident = pp.tile([P, P], bf16, tag="ident", name="ident")
            # qk_sb slots are pooled: tile jt lives in tag {q,k}{jt%4} and is
            # reclaimed ~4 pairs later, long after its heads' seg_copies.
            qk_sb = {}
            stage = [pp.tile([P, D], bf16, tag=f"st{i}", name=f"st{i}")
                     for i in range(NS)]
            vpad = [pp.tile([P, VPADW], bf16, tag=f"vp{i}", name=f"vp{i}")
                    for i in range(NS)]
            att_s = [pp.tile([P, D], bf16, tag=f"as{i}", name=f"as{i}")
                     for i in range(NS)]
            attnp = [pp.tile([P, S], bf16, tag=f"at{i}", name=f"at{i}")
                     for i in range(ND)]

            # qkw slab 0 first (small, unblocks qk pair 0 right after hT),
            # then hT (the critical 2.4MB load), then v weights.
            # SP's DMA issue rate (~0.3GB/ms of SEQ time) is the startup
            # bottleneck, so only the PE-critical loads go on SP: qk weight
            # slabs + hT (slab-by-slab so the first qk chain's dt-th matmul
            # only waits for slab dt). Everything else issues from the ACT
            # queue, which is idle until the first exp (~25us in).
            wjt0 = wsp.tile([P, D], bf16, tag="wjt", name="wjt")
            # first d-tile of the first weight slab + first half of hT slab 0
            # land first so the very first matmul can start ~1.7us in
            w9 = wsp.tile([P, D], bf16, tag="wjt", name="wjt")
            nc.sync.dma_start(wjt0[:, 0:P], qkwT_d[:, 0:P])
            nc.sync.dma_start(hT[:, 0:512], hT_d[:, 0:512])
            nc.sync.dma_start(w9[:, 0:P], qkwT_d[:, ND * D:ND * D + P])
            nc.sync.dma_start(wjt0[:, P:D], qkwT_d[:, P:D])
            nc.sync.dma_start(w9[:, P:D], qkwT_d[:, ND * D + P:(ND + 1) * D])
            nc.sync.dma_start(hT[:, 512:S], hT_d[:, 512:S])
            for dt in range(1, ND):
                nc.sync.dma_start(hT[:, dt * S:(dt + 1) * S],
                                  hT_d[:, dt * S:(dt + 1) * S])
            for hc in range(4):
                nc.scalar.dma_start(
                    vwT[:, hc * VCH:(hc + 1) * VCH],
                    vwT_d[:, hc * VCH:(hc + 1) * VCH])
            nc.scalar.dma_start(cosT[0:HD, :], cosT_d[:, :])
            nc.scalar.dma_start(sinT[0:HD, :], sinT_d[:, :])
            nc.scalar.dma_start(pwT[:], pwT_d[:, :])
            make_identity(nc, ident[:])
            # ones columns for the softmax denominator (col 72 per head)
            for st in range(NS):
                ones_col = vpad[st][:].rearrange(
                    "p (h c) -> p h c", c=VP)[:, :, HD:VP]
                nc.vector.memset(ones_col, 1.0)

            def emit_v():
                # v: out[s-tile, j-chunk] (free 288), 4 heads per chunk
                for hc in range(4):
                    for st in range(NS):
                        ps = psp.tile([P, 512], f32, tag="small", bufs=2,
                                      name="vps")
                        for dt in range(ND):
                            nc.tensor.matmul(
                                ps[:, 0:4 * HD],
                                hT[:, dt * S + st * P: dt * S + (st + 1) * P],
                                vwT[:, hc * VCH + dt * 4 * HD:
                                    hc * VCH + (dt + 1) * 4 * HD],
                                start=(dt == 0), stop=(dt == ND - 1))
                        dst = vpad[st][:].rearrange(
                            "p (h c) -> p h c", c=VP)[:, hc * 4:(hc + 1) * 4,
                                                      0:HD]
                        src = ps[:, 0:4 * HD].rearrange(
                            "p (h c) -> p h c", c=HD)
                        nc.vector.tensor_copy(dst, src)

            def seg_copy(dst_tile, dst_row, j0, n):
                while n > 0:
                    t, r = j0 // P, j0 % P
                    c = min(n, P - r)
                    nc.gpsimd.dma_start(
                        dst_tile[dst_row:dst_row + c, :],
                        qk_sb[t][r:r + c, :])
                    dst_row += c
                    j0 += c
                    n -= c

            def emit_qk_tile(jt, w=None):
                if w is None:
                    w = wsp.tile([P, D], bf16, tag="wjt", name="wjt")
                    nc.sync.dma_start(w[:], qkwT_d[:, jt * D:(jt + 1) * D])
                side = "q" if jt < ND else "k"
                qk_sb[jt] = pp.tile([P, S], bf16, tag=f"{side}{jt % 4}",
                                    name=f"qk{jt}")
                for sc in range(2):
                    ps = psp.tile([P, 512], f32, tag="small", bufs=2,
                                  name="qkps")
                    for dt in range(ND):
                        nc.tensor.matmul(
                            ps[:], w[:, dt * P:(dt + 1) * P],
                            hT[:, dt * S + sc * 512: dt * S + (sc + 1) * 512],
                            start=(dt == 0), stop=(dt == ND - 1))
                    nc.vector.tensor_copy(
                        qk_sb[jt][:, sc * 512:(sc + 1) * 512], ps[:])

            def emit_head(h, mid=None):
                qj, kj = h * HD, D + h * HD
                qh = wp.tile([P, S], bf16, tag="qh", name="qh")
                kh = wp.tile([P, S], bf16, tag="kh", name="kh")
                rq = wp.tile([P, S], bf16, tag="rq", name="rq")
                rk = wp.tile([P, S], bf16, tag="rk", name="rk")
                seg_copy(qh, 0, qj, HD)
                seg_copy(kh, 0, kj, HD)
                seg_copy(rq, 0, qj + HHD, HHD)
                seg_copy(rq, HHD, qj, HHD)
                seg_copy(rk, 0, kj + HHD, HHD)
                seg_copy(rk, HHD, kj, HHD)
                # q' = q*cos + rot(q)*sin_signed (sin rows 0:36 negated)
                nc.vector.tensor_mul(rq[0:HD, :], rq[0:HD, :], sinT[0:HD, :])
                nc.vector.tensor_mul(qh[0:HD, :], qh[0:HD, :], cosT[0:HD, :])
                nc.vector.tensor_add(qh[0:HD, :], qh[0:HD, :], rq[0:HD, :])
                nc.vector.tensor_mul(rk[0:HD, :], rk[0:HD, :], sinT[0:HD, :])
                nc.vector.tensor_mul(kh[0:HD, :], kh[0:HD, :], cosT[0:HD, :])
                nc.vector.tensor_add(kh[0:HD, :], kh[0:HD, :], rk[0:HD, :])

                # scores_T[ks, qs] + exp
                ex = [ep.tile([P, S], bf16, tag="exp", name="exp")
                      for _ in range(NS)]
                for kt in range(NS):
                    ps = psp.tile([P, S], f32, tag="big", bufs=2, name="sps")
                    for qc in range(2):
                        nc.tensor.matmul(
                            ps[:, qc * 512:(qc + 1) * 512],
                            kh[0:HD, kt * P:(kt + 1) * P],
                            qh[0:HD, qc * 512:(qc + 1) * 512],
                            start=True, stop=True)
                    nc.scalar.activation(
                        ex[kt][:], ps[:],
                        mybir.ActivationFunctionType.Exp, scale=SCALE)

                if mid is not None:
                    mid()

                # PV: out[qs, 73] accumulated over ks; denominator at col 72.
                # 4 chains share one psum bank (128-col offsets) so 8 chains
                # are in flight across 2 slots and one reciprocal serves 4 —
                # the DVE normalize then retires faster than the PE produces.
                for g in range(2):
                    ps = psp.tile([P, 512], f32, tag="pv", bufs=2,
                                  name="pvps")
                    for i in range(4):
                        qt = g * 4 + i
                        for kt in range(NS):
                            nc.tensor.matmul(
                                ps[:, i * P:i * P + VP],
                                ex[kt][:, qt * P:(qt + 1) * P],
                                vpad[kt][:, h * VP:(h + 1) * VP],
                                start=(kt == 0), stop=(kt == NS - 1))
                    rcp = wp.tile([P, 4], f32, tag="rcp", name="rcp")
                    nc.vector.reciprocal(
                        rcp[:].rearrange("p (a b) -> p a b", b=1),
                        ps[:].rearrange("p (a b) -> p a b", b=P)[:, :,
                                                                HD:VP])
                    for i in range(4):
                        qt = g * 4 + i
                        nc.vector.tensor_scalar_mul(
                            att_s[qt][:, h * HD:(h + 1) * HD],
                            ps[:, i * P:i * P + HD], rcp[:, i:i + 1])

            def emit_transpose(fc):
                # attnp[fc][f, qs] <- att_s[qt][qs, f].T for all 8 qt
                for g in range(2):
                    tp = psp.tile([P, S], bf16, tag="small", bufs=2, name="tp")
                    for i in range(4):
                        qt = g * 4 + i
                        nc.tensor.transpose(
                            tp[:, i * P:(i + 1) * P],
                            att_s[qt][:, fc * P:(fc + 1) * P],
                            ident[:])
                    nc.vector.tensor_copy(
                        attnp[fc][:, g * 512:(g + 1) * 512], tp[:, 0:512])

            def emit_p1():
                # proj partial over f-tiles 0..4 (ready after head 8),
                # staged to SBUF bf16. Fills the PE while ACT finishes the
                # last heads' exps.
                for st in range(NS):
                    for ec in range(3):
                        ps = psp.tile([P, 512], f32, tag="small", bufs=2,
                                      name="p1s")
                        for ft in range(5):
                            nc.tensor.matmul(
                                ps[:, 0:384],
                                attnp[ft][:, st * P:(st + 1) * P],
                                pwT[:, ft * D + ec * 384:
                                    ft * D + (ec + 1) * 384],
                                start=(ft == 0), stop=(ft == 4))
                        nc.vector.tensor_copy(
                            stage[st][:, ec * 384:(ec + 1) * 384],
                            ps[:, 0:384])

            # pair 0: both tiles' chains interleaved per d-slab so the
            # DMA-paced startup (hT slabs arrive every ~0.7us) feeds four
            # matmuls per slab instead of one. tile 9's chains borrow the
            # (otherwise idle at startup) "big" psum tag.
            for side, jt in (("q", 0), ("k", ND)):
                qk_sb[jt] = pp.tile([P, S], bf16, tag=f"{side}{jt % 4}",
                                    name=f"qk{jt}")
            ps0 = [psp.tile([P, 512], f32, tag="small", bufs=2, name="qkps")
                   for _ in range(2)]
            ps9 = [psp.tile([P, S], f32, tag="big", bufs=2, name="qkps9")
                   for _ in range(2)]
            for dt in range(ND):
                for w, pss in ((wjt0, ps0), (w9, ps9)):
                    for sc in range(2):
                        nc.tensor.matmul(
                            pss[sc][:, 0:512], w[:, dt * P:(dt + 1) * P],
                            hT[:, dt * S + sc * 512: dt * S + (sc + 1) * 512],
                            start=(dt == 0), stop=(dt == ND - 1))
            for jt, pss in ((0, ps0), (ND, ps9)):
                for sc in range(2):
                    nc.vector.tensor_copy(
                        qk_sb[jt][:, sc * 512:(sc + 1) * 512],
                        pss[sc][:, 0:512])
            # v right after pair 0 so head 0's redistribute+rope latency
            # hides under the v matmuls; later pairs run one group ahead of
            # their heads so the PE always has qk work buffered while a
            # head's redistribute runs on Pool/DVE.
            emit_v()
            emit_head(0)
            sched = {1: [1, 2], 2: [3, 4], 3: [5, 6], 4: [7], 5: [8, 9],
                     6: [10, 11], 7: [12, 13], 8: [14]}
            for i in range(1, ND):
                emit_qk_tile(i)
                emit_qk_tile(ND + i)
                for h in sched[i]:
                    emit_head(h)
            # last head whose att_s cols overlap f-chunk fc:
            #  fc0:h1 fc1:h3 fc2:h5 fc3:h7 fc4:h8 fc5:h10 fc6:h12 fc7:h14 fc8:h15
            for fc in range(5):
                emit_transpose(fc)

            def _mid():
                emit_p1()
                for fc in range(5, 8):
                    emit_transpose(fc)
            emit_head(15, mid=_mid)
            emit_transpose(8)

            # ---- proj pass 2: ft 5..8, P1 added back in the output copy ----
            for st in range(NS):
                for ec in range(3):
                    ps = psp.tile([P, 512], f32, tag="small", bufs=2,
                                  name="ops")
                    for ft in range(5, ND):
                        nc.tensor.matmul(
                            ps[:, 0:384], attnp[ft][:, st * P:(st + 1) * P],
                            pwT[:, ft * D + ec * 384: ft * D + (ec + 1) * 384],
                            start=(ft == 5), stop=(ft == ND - 1))
                    osb = wp.tile([P, 384], f32, tag="osb", bufs=6,
                                  name="osb")
                    nc.vector.tensor_add(
                        osb[:], ps[:, 0:384],
                        stage[st][:, ec * 384:(ec + 1) * 384])
                    nc.sync.dma_start(
                        out_d[st * P:(st + 1) * P, ec * 384:(ec + 1) * 384],
                        osb[:])

    nc.compile()
    return nc


def _get_nc():
    if "nc" not in _CACHE:
        _CACHE["nc"] = _build()
    return _CACHE["nc"]


def prep_in_maps(hidden_states, cos, sin, qkv_w, qkv_b, proj_w, proj_b):
    import ml_dtypes

    bf = ml_dtypes.bfloat16
    hidden_states = np.asarray(hidden_states, dtype=np.float32)
    cos = np.asarray(cos, dtype=np.float32)
    sin = np.asarray(sin, dtype=np.float32)
    qkv_w = np.asarray(qkv_w, dtype=np.float32)
    qkv_b = np.asarray(qkv_b, dtype=np.float32)
    proj_w = np.asarray(proj_w, dtype=np.float32)

    assert np.abs(qkv_b).max() == 0.0, "nonzero qkv_b not supported"

    cosT = np.ascontiguousarray(cos.T).astype(bf)                 # [72, 1024]
    sinT = np.ascontiguousarray(sin.T)
    sinT = np.concatenate([-sinT[:HHD], sinT[HHD:]], 0).astype(bf)

    # qk weights: [128, (jt, dt, 128)] so each j-tile is one contiguous slab
    qkwT = np.ascontiguousarray(qkv_w[:NQK].T)                    # [1152, 2304]
    qkw_t = np.ascontiguousarray(
        qkwT.reshape(ND, P, NJQK, P).transpose(1, 2, 0, 3).reshape(
            P, NJQK * D)).astype(bf)
    # v weights: [128, (hc, dt, 288)]
    vwT = np.ascontiguousarray(qkv_w[NQK:].T)                     # [1152, 1152]
    vw_t = np.ascontiguousarray(
        vwT.reshape(ND, P, 4, 4 * HD).transpose(1, 2, 0, 3).reshape(
            P, ND * D)).astype(bf)
    # proj weights: [128, (ft, 1152)]
    pwT = np.ascontiguousarray(proj_w.T)                          # [1152, 1152]
    pw_t = np.ascontiguousarray(
        pwT.reshape(ND, P, D).transpose(1, 0, 2).reshape(P, ND * D)).astype(bf)

    in_maps = []
    for b in range(NCORES):
        hTb = np.ascontiguousarray(hidden_states[b].T)            # [1152, 1024]
        hT_t = np.ascontiguousarray(
            hTb.reshape(ND, P, S).transpose(1, 0, 2).reshape(
                P, ND * S)).astype(bf)
        in_maps.append({
            "hT": hT_t,
            "cosT": cosT, "sinT": sinT,
            "qkwT": qkw_t, "vwT": vw_t, "pwT": pw_t,
        })

    return in_maps


def kernel(hidden_states, cos, sin, qkv_w, qkv_b, proj_w, proj_b,
           _profile=False):
    from concourse.bass_utils import run_bass_kernel_spmd

    proj_b = np.asarray(proj_b, dtype=np.float32)
    in_maps = prep_in_maps(hidden_states, cos, sin, qkv_w, qkv_b,
                           proj_w, proj_b)
    nc = _get_nc()
    res = run_bass_kernel_spmd(nc, in_maps, core_ids=list(range(NCORES)),
                               trace=_profile)
    _CACHE["last_exec_time_ns"] = res.exec_time_ns
    out = np.stack([np.asarray(res.results[b]["out"], dtype=np.float32)
                    for b in range(NCORES)])
    return out + proj_b[None, None, :]


# revision 46
# speedup vs baseline: 1.0030x; 1.0030x over previous
"""SigLIP2 attention block on 8 TRN2 NeuronCores.

Strategy: data-parallel over batch (B=8 -> 1 batch element per core, no
collectives). All weights pre-transposed/pre-tiled + pre-cast to bf16 on the
host so the on-chip kernel is a pure matmul + softmax pipeline. Matmul time
on the PE is proportional to the OUTPUT FREE SIZE only, so every matmul is
oriented with the big dims on partitions and the small dims on free:

  per core (batch b):
    v:      psum[s,j] = hT[d,s].T @ v_wT[d,j]           (free 288)
    qkv:    q/k psum[j,s] = qk_wT[d,j].T @ hT[d,s]      (free 512, j-major)
    rope:   per-head redistribute (partition-shifted SBUF DMA on gpsimd) +
            rot-half via shifted copies, then q' = q*cosT + rot(q)*sinT_signed
            on DVE (in place)
    attn:   scores_T[ks,qs] = k'h[hd,ks].T @ q'h[hd,qs]  (K=72, free 512)
            exp on ACT (scale=1/sqrt(72), no max-subtract: |scores| is O(1))
            PV: attn[qs,hd+1] accumulated over ks tiles with ex[ks,qs] as the
            stationary operand (free 73!). vpad has a ones column at 72 so
            the softmax denominator lands in free col 72; normalize with
            reciprocal + tensor_scalar (per-partition scalar) on DVE into
            qs-major att_s tiles.
    trans:  att_s[qs,f] -> attnp[f,qs] via PE identity-matmul transposes
            (4 qt chunks per psum tile, one DVE copy per group)
    proj:   out[s,e] = attnp[f,s].T @ proj_wT[f,e] (free 384), split into
            P1 (f-tiles 0..4, staged to SBUF bf16, emitted inside head 15's
            scores->PV window to fill the ACT-bound stretch) and P2
            (f-tiles 5..8; P1 added back in the DVE output copy)
  proj_b added on host (linear); qkv_b is all-zero in this problem (asserted).

  Schedule: pair-0 qk tiles interleave per hT d-slab during the DMA-paced
  startup; v runs right after pair 0 (hides head 0's redistribute+rope
  latency); later pairs alternate with the heads they complete. Bulk weight
  loads issue from the ACT queue (idle until the first exp) because SP's
  DMA-issue rate is the startup bottleneck.
"""

import os
import sys
import numpy as np

sys.path.insert(0, "/opt/trn_rl_repo")

B, S, D = 8, 1024, 1152
H, HD = 16, 72
HHD = HD // 2  # 36
NQK = 2 * D    # 2304 q+k rows
P = 128
NCORES = 8
SCALE = float(HD) ** -0.5

ND = D // P      # 9 d tiles
NS = S // P      # 8 s tiles
NJQK = NQK // P  # 18 qk j tiles
VP = HD + 1      # 73: head dim + denominator col
VPADW = H * VP   # 1168

_CACHE = {}


def _build():
    import concourse.bass as bass
    import concourse.bacc as bacc
    import concourse.mybir as mybir
    from concourse import tile
    from concourse.masks import make_identity

    bf16 = mybir.dt.bfloat16
    f32 = mybir.dt.float32

    nc = bacc.Bacc(None)

    # host-pretiled inputs (see prep_in_maps)
    hT_d = nc.declare_dram_parameter("hT", [P, ND * S], bf16, isOutput=False)
    cosT_d = nc.declare_dram_parameter("cosT", [HD, S], bf16, isOutput=False)
    sinT_d = nc.declare_dram_parameter("sinT", [HD, S], bf16, isOutput=False)
    qkwT_d = nc.declare_dram_parameter("qkwT", [P, NJQK * D], bf16,
                                       isOutput=False)
    vwT_d = nc.declare_dram_parameter("vwT", [P, ND * D], bf16, isOutput=False)
    pwT_d = nc.declare_dram_parameter("pwT", [P, ND * D], bf16, isOutput=False)
    out_d = nc.declare_dram_parameter("out", [S, D], f32, isOutput=True)

    VCH = 4 * HD * ND  # 2592 cols per vw hc-chunk

    with tile.TileContext(nc) as tc:
        with (
            tc.tile_pool(name="persist", bufs=1) as pp,
            tc.tile_pool(name="wstream", bufs=3) as wsp,
            tc.tile_pool(name="work", bufs=2) as wp,
            tc.tile_pool(name="expp", bufs=10) as ep,
            tc.tile_pool(name="psp", bufs=2, space="PSUM") as psp,
        ):
            # ---- resident allocations ----
            hT = pp.tile([P, ND * S], bf16, tag="hT", name="hT")
            vwT = pp.tile([P, ND * D], bf16, tag="vwT", name="vwT")
            pwT = pp.tile([P, ND * D], bf16, tag="pwT", name="pwT")
            cosT = pp.tile([P, S], bf16, tag="cosT", name="cosT")
            sinT = pp.tile([P, S], bf16, tag="sinT", name="sinT")
            ident = pp.tile([P, P], bf16, tag="ident", name="ident")
            # qk_sb slots are pooled: tile jt lives in tag {q,k}{jt%4} and is
            # reclaimed ~4 pairs later, long after its heads' seg_copies.
            qk_sb = {}
            stage = [pp.tile([P, D], bf16, tag=f"st{i}", name=f"st{i}")
                     for i in range(NS)]
            vpad = [pp.tile([P, VPADW], bf16, tag=f"vp{i}", name=f"vp{i}")
                    for i in range(NS)]
            att_s = [pp.tile([P, D], bf16, tag=f"as{i}", name=f"as{i}")
                     for i in range(NS)]
            attnp = [pp.tile([P, S], bf16, tag=f"at{i}", name=f"at{i}")
                     for i in range(ND)]

            # qkw slab 0 first (small, unblocks qk pair 0 right after hT),
            # then hT (the critical 2.4MB load), then v weights.
            # SP's DMA issue rate (~0.3GB/ms of SEQ time) is the startup
            # bottleneck, so only the PE-critical loads go on SP: qk weight
            # slabs + hT (slab-by-slab so the first qk chain's dt-th matmul
            # only waits for slab dt). Everything else issues from the ACT
            # queue, which is idle until the first exp (~25us in).
            wjt0 = wsp.tile([P, D], bf16, tag="wjt", name="wjt")
            # first d-tile of the first weight slab + first half of hT slab 0
            # land first so the very first matmul can start ~1.7us in
            w9 = wsp.tile([P, D], bf16, tag="wjt", name="wjt")
            nc.sync.dma_start(wjt0[:, 0:P], qkwT_d[:, 0:P])
            nc.sync.dma_start(hT[:, 0:512], hT_d[:, 0:512])
            nc.sync.dma_start(w9[:, 0:P], qkwT_d[:, ND * D:ND * D + P])
            nc.sync.dma_start(hT[:, 512:S], hT_d[:, 512:S])
            nc.sync.dma_start(wjt0[:, P:D], qkwT_d[:, P:D])
            nc.sync.dma_start(w9[:, P:D], qkwT_d[:, ND * D + P:(ND + 1) * D])
            for dt in range(1, ND):
                nc.sync.dma_start(hT[:, dt * S:(dt + 1) * S],
                                  hT_d[:, dt * S:(dt + 1) * S])
            for hc in range(4):
                nc.scalar.dma_start(
                    vwT[:, hc * VCH:(hc + 1) * VCH],
                    vwT_d[:, hc * VCH:(hc + 1) * VCH])
            nc.scalar.dma_start(cosT[0:HD, :], cosT_d[:, :])
            nc.scalar.dma_start(sinT[0:HD, :], sinT_d[:, :])
            nc.scalar.dma_start(pwT[:], pwT_d[:, :])
            make_identity(nc, ident[:])
            # ones columns for the softmax denominator (col 72 per head)
            for st in range(NS):
                ones_col = vpad[st][:].rearrange(
                    "p (h c) -> p h c", c=VP)[:, :, HD:VP]
                nc.vector.memset(ones_col, 1.0)

            def emit_v():
                # v: out[s-tile, j-chunk] (free 288), 4 heads per chunk
                for hc in range(4):
                    for st in range(NS):
                        ps = psp.tile([P, 512], f32, tag="small", bufs=2,
                                      name="vps")
                        for dt in range(ND):
                            nc.tensor.matmul(
                                ps[:, 0:4 * HD],
                                hT[:, dt * S + st * P: dt * S + (st + 1) * P],
                                vwT[:, hc * VCH + dt * 4 * HD:
                                    hc * VCH + (dt + 1) * 4 * HD],
                                start=(dt == 0), stop=(dt == ND - 1))
                        dst = vpad[st][:].rearrange(
                            "p (h c) -> p h c", c=VP)[:, hc * 4:(hc + 1) * 4,
                                                      0:HD]
                        src = ps[:, 0:4 * HD].rearrange(
                            "p (h c) -> p h c", c=HD)
                        nc.vector.tensor_copy(dst, src)

            def seg_copy(dst_tile, dst_row, j0, n):
                while n > 0:
                    t, r = j0 // P, j0 % P
                    c = min(n, P - r)
                    nc.gpsimd.dma_start(
                        dst_tile[dst_row:dst_row + c, :],
                        qk_sb[t][r:r + c, :])
                    dst_row += c
                    j0 += c
                    n -= c

            def emit_qk_tile(jt, w=None):
                if w is None:
                    w = wsp.tile([P, D], bf16, tag="wjt", name="wjt")
                    nc.sync.dma_start(w[:], qkwT_d[:, jt * D:(jt + 1) * D])
                side = "q" if jt < ND else "k"
                qk_sb[jt] = pp.tile([P, S], bf16, tag=f"{side}{jt % 4}",
                                    name=f"qk{jt}")
                for sc in range(2):
                    ps = psp.tile([P, 512], f32, tag="small", bufs=2,
                                  name="qkps")
                    for dt in range(ND):
                        nc.tensor.matmul(
                            ps[:], w[:, dt * P:(dt + 1) * P],
                            hT[:, dt * S + sc * 512: dt * S + (sc + 1) * 512],
                            start=(dt == 0), stop=(dt == ND - 1))
                    nc.vector.tensor_copy(
                        qk_sb[jt][:, sc * 512:(sc + 1) * 512], ps[:])

            def emit_head(h, mid=None):
                qj, kj = h * HD, D + h * HD
                qh = wp.tile([P, S], bf16, tag="qh", name="qh")
                kh = wp.tile([P, S], bf16, tag="kh", name="kh")
                rq = wp.tile([P, S], bf16, tag="rq", name="rq")
                rk = wp.tile([P, S], bf16, tag="rk", name="rk")
                seg_copy(qh, 0, qj, HD)
                seg_copy(kh, 0, kj, HD)
                seg_copy(rq, 0, qj + HHD, HHD)
                seg_copy(rq, HHD, qj, HHD)
                seg_copy(rk, 0, kj + HHD, HHD)
                seg_copy(rk, HHD, kj, HHD)
                # q' = q*cos + rot(q)*sin_signed (sin rows 0:36 negated)
                nc.vector.tensor_mul(rq[0:HD, :], rq[0:HD, :], sinT[0:HD, :])
                nc.vector.tensor_mul(qh[0:HD, :], qh[0:HD, :], cosT[0:HD, :])
                nc.vector.tensor_add(qh[0:HD, :], qh[0:HD, :], rq[0:HD, :])
                nc.vector.tensor_mul(rk[0:HD, :], rk[0:HD, :], sinT[0:HD, :])
                nc.vector.tensor_mul(kh[0:HD, :], kh[0:HD, :], cosT[0:HD, :])
                nc.vector.tensor_add(kh[0:HD, :], kh[0:HD, :], rk[0:HD, :])

                # scores_T[ks, qs] + exp
                ex = [ep.tile([P, S], bf16, tag="exp", name="exp")
                      for _ in range(NS)]
                for kt in range(NS):
                    ps = psp.tile([P, S], f32, tag="big", bufs=2, name="sps")
                    for qc in range(2):
                        nc.tensor.matmul(
                            ps[:, qc * 512:(qc + 1) * 512],
                            kh[0:HD, kt * P:(kt + 1) * P],
                            qh[0:HD, qc * 512:(qc + 1) * 512],
                            start=True, stop=True)
                    nc.scalar.activation(
                        ex[kt][:], ps[:],
                        mybir.ActivationFunctionType.Exp, scale=SCALE)

                if mid is not None:
                    mid()

                # PV: out[qs, 73] accumulated over ks; denominator at col 72.
                # 4 chains share one psum bank (128-col offsets) so 8 chains
                # are in flight across 2 slots and one reciprocal serves 4 —
                # the DVE normalize then retires faster than the PE produces.
                for g in range(2):
                    ps = psp.tile([P, 512], f32, tag="pv", bufs=2,
                                  name="pvps")
                    for i in range(4):
                        qt = g * 4 + i
                        for kt in range(NS):
                            nc.tensor.matmul(
                                ps[:, i * P:i * P + VP],
                                ex[kt][:, qt * P:(qt + 1) * P],
                                vpad[kt][:, h * VP:(h + 1) * VP],
                                start=(kt == 0), stop=(kt == NS - 1))
                    rcp = wp.tile([P, 4], f32, tag="rcp", name="rcp")
                    nc.vector.reciprocal(
                        rcp[:].rearrange("p (a b) -> p a b", b=1),
                        ps[:].rearrange("p (a b) -> p a b", b=P)[:, :,
                                                                HD:VP])
                    for i in range(4):
                        qt = g * 4 + i
                        nc.vector.tensor_scalar_mul(
                            att_s[qt][:, h * HD:(h + 1) * HD],
                            ps[:, i * P:i * P + HD], rcp[:, i:i + 1])

            def emit_transpose(fc):
                # attnp[fc][f, qs] <- att_s[qt][qs, f].T for all 8 qt
                for g in range(2):
                    tp = psp.tile([P, S], bf16, tag="small", bufs=2, name="tp")
                    for i in range(4):
                        qt = g * 4 + i
                        nc.tensor.transpose(
                            tp[:, i * P:(i + 1) * P],
                            att_s[qt][:, fc * P:(fc + 1) * P],
                            ident[:])
                    nc.vector.tensor_copy(
                        attnp[fc][:, g * 512:(g + 1) * 512], tp[:, 0:512])

            def emit_p1():
                # proj partial over f-tiles 0..4 (ready after head 8),
                # staged to SBUF bf16. Fills the PE while ACT finishes the
                # last heads' exps.
                for st in range(NS):
                    for ec in range(3):
                        ps = psp.tile([P, 512], f32, tag="small", bufs=2,
                                      name="p1s")
                        for ft in range(5):
                            nc.tensor.matmul(
                                ps[:, 0:384],
                                attnp[ft][:, st * P:(st + 1) * P],
                                pwT[:, ft * D + ec * 384:
                                    ft * D + (ec + 1) * 384],
                                start=(ft == 0), stop=(ft == 4))
                        nc.vector.tensor_copy(
                            stage[st][:, ec * 384:(ec + 1) * 384],
                            ps[:, 0:384])

            # pair 0: both tiles' chains interleaved per d-slab so the
            # DMA-paced startup (hT slabs arrive every ~0.7us) feeds four
            # matmuls per slab instead of one. tile 9's chains borrow the
            # (otherwise idle at startup) "big" psum tag.
            for side, jt in (("q", 0), ("k", ND)):
                qk_sb[jt] = pp.tile([P, S], bf16, tag=f"{side}{jt % 4}",
                                    name=f"qk{jt}")
            ps0 = [psp.tile([P, 512], f32, tag="small", bufs=2, name="qkps")
                   for _ in range(2)]
            ps9 = [psp.tile([P, S], f32, tag="big", bufs=2, name="qkps9")
                   for _ in range(2)]
            for dt in range(ND):
                for w, pss in ((wjt0, ps0), (w9, ps9)):
                    for sc in range(2):
                        nc.tensor.matmul(
                            pss[sc][:, 0:512], w[:, dt * P:(dt + 1) * P],
                            hT[:, dt * S + sc * 512: dt * S + (sc + 1) * 512],
                            start=(dt == 0), stop=(dt == ND - 1))
            for jt, pss in ((0, ps0), (ND, ps9)):
                for sc in range(2):
                    nc.vector.tensor_copy(
                        qk_sb[jt][:, sc * 512:(sc + 1) * 512],
                        pss[sc][:, 0:512])
            # v right after pair 0 so head 0's redistribute+rope latency
            # hides under the v matmuls; later pairs run one group ahead of
            # their heads so the PE always has qk work buffered while a
            # head's redistribute runs on Pool/DVE.
            emit_v()
            emit_head(0)
            sched = {1: [1, 2], 2: [3, 4], 3: [5, 6], 4: [7], 5: [8, 9],
                     6: [10, 11], 7: [12, 13], 8: [14]}
            for i in range(1, ND):
                emit_qk_tile(i)
                emit_qk_tile(ND + i)
                for h in sched[i]:
                    emit_head(h)
            # last head whose att_s cols overlap f-chunk fc:
            #  fc0:h1 fc1:h3 fc2:h5 fc3:h7 fc4:h8 fc5:h10 fc6:h12 fc7:h14 fc8:h15
            for fc in range(5):
                emit_transpose(fc)

            def _mid():
                emit_p1()
                for fc in range(5, 8):
                    emit_transpose(fc)
            emit_head(15, mid=_mid)
            emit_transpose(8)

            # ---- proj pass 2: ft 5..8, P1 added back in the output copy ----
            for st in range(NS):
                for ec in range(3):
                    ps = psp.tile([P, 512], f32, tag="small", bufs=2,
                                  name="ops")
                    for ft in range(5, ND):
                        nc.tensor.matmul(
                            ps[:, 0:384], attnp[ft][:, st * P:(st + 1) * P],
                            pwT[:, ft * D + ec * 384: ft * D + (ec + 1) * 384],
                            start=(ft == 5), stop=(ft == ND - 1))
                    osb = wp.tile([P, 384], f32, tag="osb", bufs=6,
                                  name="osb")
                    nc.vector.tensor_add(
                        osb[:], ps[:, 0:384],
                        stage[st][:, ec * 384:(ec + 1) * 384])
                    nc.sync.dma_start(
                        out_d[st * P:(st + 1) * P, ec * 384:(ec + 1) * 384],
                        osb[:])

    nc.compile()
    return nc


def _get_nc():
    if "nc" not in _CACHE:
        _CACHE["nc"] = _build()
    return _CACHE["nc"]


def prep_in_maps(hidden_states, cos, sin, qkv_w, qkv_b, proj_w, proj_b):
    import ml_dtypes

    bf = ml_dtypes.bfloat16
    hidden_states = np.asarray(hidden_states, dtype=np.float32)
    cos = np.asarray(cos, dtype=np.float32)
    sin = np.asarray(sin, dtype=np.float32)
    qkv_w = np.asarray(qkv_w, dtype=np.float32)
    qkv_b = np.asarray(qkv_b, dtype=np.float32)
    proj_w = np.asarray(proj_w, dtype=np.float32)

    assert np.abs(qkv_b).max() == 0.0, "nonzero qkv_b not supported"

    cosT = np.ascontiguousarray(cos.T).astype(bf)                 # [72, 1024]
    sinT = np.ascontiguousarray(sin.T)
    sinT = np.concatenate([-sinT[:HHD], sinT[HHD:]], 0).astype(bf)

    # qk weights: [128, (jt, dt, 128)] so each j-tile is one contiguous slab
    qkwT = np.ascontiguousarray(qkv_w[:NQK].T)                    # [1152, 2304]
    qkw_t = np.ascontiguousarray(
        qkwT.reshape(ND, P, NJQK, P).transpose(1, 2, 0, 3).reshape(
            P, NJQK * D)).astype(bf)
    # v weights: [128, (hc, dt, 288)]
    vwT = np.ascontiguousarray(qkv_w[NQK:].T)                     # [1152, 1152]
    vw_t = np.ascontiguousarray(
        vwT.reshape(ND, P, 4, 4 * HD).transpose(1, 2, 0, 3).reshape(
            P, ND * D)).astype(bf)
    # proj weights: [128, (ft, 1152)]
    pwT = np.ascontiguousarray(proj_w.T)                          # [1152, 1152]
    pw_t = np.ascontiguousarray(
        pwT.reshape(ND, P, D).transpose(1, 0, 2).reshape(P, ND * D)).astype(bf)

    in_maps = []
    for b in range(NCORES):
        hTb = np.ascontiguousarray(hidden_states[b].T)            # [1152, 1024]
        hT_t = np.ascontiguousarray(
            hTb.reshape(ND, P, S).transpose(1, 0, 2).reshape(
                P, ND * S)).astype(bf)
        in_maps.append({
            "hT": hT_t,
            "cosT": cosT, "sinT": sinT,
            "qkwT": qkw_t, "vwT": vw_t, "pwT": pw_t,
        })

    return in_maps


def kernel(hidden_states, cos, sin, qkv_w, qkv_b, proj_w, proj_b,
           _profile=False):
    from concourse.bass_utils import run_bass_kernel_spmd

    proj_b = np.asarray(proj_b, dtype=np.float32)
    in_maps = prep_in_maps(hidden_states, cos, sin, qkv_w, qkv_b,
                           proj_w, proj_b)
    nc = _get_nc()
    res = run_bass_kernel_spmd(nc, in_maps, core_ids=list(range(NCORES)),
                               trace=_profile)
    _CACHE["last_exec_time_ns"] = res.exec_time_ns
    out = np.stack([np.asarray(res.results[b]["out"], dtype=np.float32)
                    for b in range(NCORES)])
    return out + proj_b[None, None, :]
